# revision 38
# baseline (speedup 1.0000x reference)
"""Trainium2 Bass kernel for nn_ConvLogicLayer.

Computes y[n,c,oy,ox,p] = k0 + ka*A + kb*B + kab*A*B where A/B are
shifted-window gathers of input channels (per the packed `selection`),
and k* are per-(c,p) coefficients derived from softmax(weights) @ OP_COEFFS.

Strategy:
  - Shard C_out (512) across 8 cores -> 64 output channels per core.
  - Each core gets a specialized program: the gather indices and the
    coefficients are baked into the instruction stream (static access
    patterns + immediate scalars), so the kernel is pure streaming
    elementwise work with zero gather traffic.
  - SBUF layout: partition q = n*4 + oyblk (32 images x 4 row-blocks),
    free dim = all 64 input channels x 10 halo rows x 34 padded cols.
    A shifted 8x32 window for any (ch,ry,rx) is then a single static
    3D access pattern on one SBUF tile.
  - Per (c,p) pair: u = kab*B + ka (ScalarE), v = kb*B + k0 and
    y = w + v load-balanced across VectorE/ScalarE/GPSIMD, w = u*A
    (VectorE).  y is written p-interleaved so the per-channel output DMA
    (512KB) is 4KB-contiguous in HBM.  Input load is chunked and ordered
    by first use so compute overlaps the streaming load.
    Load/compute overlap: 2-chunk load (most-used input channels first),
    output channels ordered by ready-pair count with ready pairs emitted
    first, and the chunk split auto-tuned per core via TimelineSim.
    Cost-model estimate: 167.7us (slowest core); measured rel err on
    hardware vs the f32 reference: 3.4e-07.
"""

import os
import sys
import threading

import numpy as np

for _p in ("/opt/trn_rl_repo",):
    if _p not in sys.path and os.path.isdir(_p):
        sys.path.insert(0, _p)

import concourse.bass as bass
import concourse.bacc as bacc
import concourse.mybir as mybir
from concourse.tile import TileContext
from concourse.masks import make_identity
from concourse import bass_utils

# Problem constants (hardcoded per spec)
N, C_IN, H, W = 32, 64, 32, 32
C_OUT, KPAIRS = 512, 4
N_CORES = 8
CPC = C_OUT // N_CORES  # channels per core

P = 128          # partitions = (n=32) x (oyblk=4)
OYB = 4          # oy blocks per image
OYS = 8          # oy rows per block
HALO = 10        # rows stored per block (8 + 2 halo)
W34 = 34         # padded width
CHSZ = HALO * W34           # 340 elems per (q, channel)
XFREE = C_IN * CHSZ         # 21760 elems per partition
OUT_CSTRIDE = H * W * KPAIRS          # 4096
OUT_NSTRIDE = CPC * OUT_CSTRIDE       # 262144

OP_COEFFS = np.array([
    [0.0, 0.0, 0.0, 0.0], [0.0, 0.0, 0.0, 1.0], [0.0, 1.0, 0.0, -1.0],
    [0.0, 1.0, 0.0, 0.0], [0.0, 0.0, 1.0, -1.0], [0.0, 0.0, 1.0, 0.0],
    [0.0, 1.0, 1.0, -2.0], [0.0, 1.0, 1.0, -1.0], [1.0, -1.0, -1.0, 1.0],
    [1.0, -1.0, -1.0, 2.0], [1.0, 0.0, -1.0, 0.0], [1.0, 0.0, -1.0, 1.0],
    [1.0, -1.0, 0.0, 0.0], [1.0, -1.0, 0.0, 1.0], [1.0, 0.0, 0.0, -1.0],
    [1.0, 0.0, 0.0, 0.0],
], dtype=np.float64)

MULT = mybir.AluOpType.mult
ADD = mybir.AluOpType.add
COPY = mybir.ActivationFunctionType.Copy

# Cost-model ns for load balancing (f32, [128, 256] tiles)
DVE_TT = 327.0   # tensor_tensor, 1x
DVE_TS = 194.0   # tensor_scalar, 2x_2P
ACT_TS = 507.0   # activation, 1x + 352cyc overhead
GPS_TS = 600.0   # gpsimd tensor_scalar (sw impl efficiency ~0.6)
GPS_TT = 600.0   # gpsimd tensor_tensor (sw impl efficiency ~0.42)
PE_U = 852.0     # two f32 matmuls (identity copy + bias row) on TensorE

# Tuning knobs (A/B'd via TimelineSim; best found = ~168us slowest core)
CFG = {
    "use_gps": True,     # offload v/y ops to GPSIMD
    "tp_bufs": 6,
    "yc_bufs": 8,
    "u_act_only": True,  # u always on ScalarE
    "w_dve_only": True,  # w always on VectorE
    "load_chunk_ch": 32,  # 2-chunk load (top-used channels first)
    "load_cascade": None,
    "any_uv": False,
    # TensorE u-offload (identity matmul + bias row, u = B + ka/kab, kab
    # re-applied at the y STT): numerically exact but model-NEGATIVE -- f32
    # matmul runs at 4 cycles/row plus cold p-state, and the PE->PSUM->DVE
    # chain serializes; every tested fraction lost ~7us. Kept for reference.
    "u_pe": False,
    "kab_min": 1e-3,     # |kab| guard for the u_pe refactoring
}

last_results = [None] * N_CORES  # BassKernelResults per core (for profiling)
last_model_ns = [None] * N_CORES  # per-core TimelineSim estimate of the shipped program


def build_core_program(core, ch, ry, rx, coef):
    """One specialized Bass program for `core` (channels core*CPC..+CPC)."""
    nc = bacc.Bacc("TRN2", target_bir_lowering=False)
    xh_d = nc.dram_tensor("xh", [P, XFREE], mybir.dt.float32, kind="ExternalInput")
    kap_d = nc.dram_tensor(
        "kap", [P, CPC * KPAIRS], mybir.dt.float32, kind="ExternalInput"
    )
    out_d = nc.dram_tensor(
        "out", [N, CPC, H, W, KPAIRS], mybir.dt.float32, kind="ExternalOutput"
    )

    use_gps = CFG["use_gps"]
    use_pe = CFG.get("u_pe")
    kab_min = CFG.get("kab_min", 1e-3)
    eng_ns = {"dve": 0.0, "act": 0.0, "gps": 0.0, "pe": 0.0}

    with TileContext(nc) as tc:
        with (
            tc.tile_pool(name="xp", bufs=1) as xpool,
            tc.tile_pool(name="tp", bufs=CFG["tp_bufs"]) as tpool,
            tc.tile_pool(name="yp", bufs=CFG["yc_bufs"]) as ypool,
            tc.tile_pool(name="pp", bufs=4, space="PSUM") as ppool,
        ):
            xh = xpool.tile([P, XFREE], mybir.dt.float32)
            # Jointly order output-channel processing (greedy: next output
            # needing fewest not-yet-loaded inputs) and stream input-channel
            # loads in that discovery order, so compute starts after a couple
            # of small chunks and fully overlaps the rest of the load.
            # (Tile's subtile tracking scopes each pair's waits to the load
            # DMAs it actually reads; Bacc splits any multi-wait syncs.)
            # Two-chunk load: chunk1 = the 32 most-used input channels, then
            # the rest.  A pair only needs its 2 input channels, so ~25-35%
            # of pairs are ready after chunk1 (~16us in); channels are ordered
            # by ready-pair count and ready pairs emitted first, so compute
            # overlaps the chunk2 load.  (Tile's subtile tracking scopes each
            # pair's waits to the load DMAs it reads.)
            chunk_ch = CFG.get("load_chunk_ch", 0)
            pair_chs = {
                (cl, p4): (int(ch[core * CPC + cl, 2 * p4]), int(ch[core * CPC + cl, 2 * p4 + 1]))
                for cl in range(CPC)
                for p4 in range(KPAIRS)
            }
            if chunk_ch <= 0:
                cl_order = list(range(CPC))
                p4_order = {cl: list(range(KPAIRS)) for cl in range(CPC)}
                nc.sync.dma_start(xh[:], xh_d[:])
            else:
                use_cnt = [0] * C_IN
                for a, b in pair_chs.values():
                    use_cnt[a] += 1
                    use_cnt[b] += 1
                by_use = sorted(range(C_IN), key=lambda i: -use_cnt[i])
                sizes = CFG.get("load_cascade") or [chunk_ch, C_IN - chunk_ch]
                tier_of = {}
                pos = 0
                groups = []
                for t, sz in enumerate(sizes):
                    grp = by_use[pos : pos + sz]
                    pos += sz
                    for cch in grp:
                        tier_of[cch] = t
                    if grp:
                        groups.append(sorted(grp))
                ptier = {
                    (cl, p4): max(tier_of[a], tier_of[b])
                    for (cl, p4), (a, b) in pair_chs.items()
                }
                tiers = {cl: sorted(ptier[(cl, p4)] for p4 in range(KPAIRS)) for cl in range(CPC)}
                cl_order = sorted(range(CPC), key=lambda cl: tiers[cl])
                p4_order = {
                    cl: sorted(range(KPAIRS), key=lambda p4: ptier[(cl, p4)])
                    for cl in range(CPC)
                }
                for grp in groups:
                    run = [grp[0]]
                    for cch in grp[1:] + [None]:
                        if cch is not None and cch == run[-1] + 1:
                            run.append(cch)
                            continue
                        lo, hi = run[0] * CHSZ, (run[-1] + 1) * CHSZ
                        nc.sync.dma_start(xh[:, lo:hi], xh_d[:, lo:hi])
                        if cch is not None:
                            run = [cch]
            base = xh[:]
            pitch = base.ap[0][0]
            tens = base.tensor
            base_off = base.offset

            if use_pe:
                ident = xpool.tile([P, P], mybir.dt.float32, tag="ident")
                ones = xpool.tile([1, W * OYS], mybir.dt.float32, tag="ones")
                kap = xpool.tile([P, CPC * KPAIRS], mybir.dt.float32, tag="kap")
                make_identity(nc, ident[:])
                nc.vector.memset(ones[:], 1.0)
                nc.sync.dma_start(kap[:], kap_d[:])

            for cl in cl_order:
                c = core * CPC + cl
                yc = ypool.tile([P, OYS * W * KPAIRS], mybir.dt.float32, tag="yc")
                ybase = yc[:]
                ypitch = ybase.ap[0][0]
                for p4 in p4_order[cl]:
                    ka_, kb_ = 2 * p4, 2 * p4 + 1
                    offA = base_off + int(ch[c, ka_]) * CHSZ + int(ry[c, ka_]) * W34 + int(rx[c, ka_])
                    offB = base_off + int(ch[c, kb_]) * CHSZ + int(ry[c, kb_]) * W34 + int(rx[c, kb_])
                    A_ap = bass.AP(tens, offA, [[pitch, P], [W34, OYS], [1, W]])
                    B_ap = bass.AP(tens, offB, [[pitch, P], [W34, OYS], [1, W]])

                    k0 = float(coef[c, p4, 0])
                    ka = float(coef[c, p4, 1])
                    kb = float(coef[c, p4, 2])
                    kab = float(coef[c, p4, 3])

                    u = tpool.tile([P, OYS * W], mybir.dt.float32, tag="u")
                    v = tpool.tile([P, OYS * W], mybir.dt.float32, tag="v")
                    w = tpool.tile([P, OYS * W], mybir.dt.float32, tag="w")
                    u3 = u[:].rearrange("p (a b) -> p a b", b=W)
                    v3 = v[:].rearrange("p (a b) -> p a b", b=W)
                    w3 = w[:].rearrange("p (a b) -> p a b", b=W)

                    def pick(cands):
                        eng, cost = min(cands, key=lambda c: eng_ns[c[0]] + c[1])
                        eng_ns[eng] += cost
                        return eng

                    # u = kab*B + ka
                    pe_ok = use_pe and abs(kab) >= kab_min
                    ueng = None
                    if CFG.get("any_uv"):
                        nc.any.tensor_scalar(u3, B_ap, kab, ka, MULT, ADD)
                        nc.any.tensor_scalar(v3, B_ap, kb, k0, MULT, ADD)
                        eng_ns["act"] += ACT_TS  # rough accounting
                        eng_ns["dve"] += DVE_TS
                    else:
                        if CFG.get("u_act_only"):
                            ucands = [("act", ACT_TS)]
                        else:
                            ucands = [("act", ACT_TS), ("dve", DVE_TS)]
                            if use_gps:
                                ucands.append(("gps", GPS_TS))
                        if pe_ok:
                            ucands = ucands + [("pe", PE_U)]
                        ueng = pick(ucands)
                        if ueng == "pe":
                            # u = B + ka/kab via identity matmul + bias row;
                            # kab is re-applied at the y step (STT below).
                            upsum = ppool.tile([P, OYS * W], mybir.dt.float32, tag="up")
                            j = cl * KPAIRS + p4
                            nc.tensor.matmul(
                                out=upsum[:], lhsT=ident[:], rhs=B_ap,
                                start=True, stop=False,
                            )
                            nc.tensor.matmul(
                                out=upsum[:],
                                lhsT=kap[0:1, j : j + 1].to_broadcast((1, P)),
                                rhs=ones[:], start=False, stop=True,
                            )
                            u3 = upsum[:].rearrange("p (a b) -> p a b", b=W)
                        elif ueng == "act":
                            nc.scalar.activation(u3, B_ap, COPY, bias=ka, scale=kab)
                        elif ueng == "gps":
                            nc.gpsimd.tensor_scalar(u3, B_ap, kab, ka, MULT, ADD)
                        else:
                            nc.vector.tensor_scalar(u3, B_ap, kab, ka, MULT, ADD)
                        # v = kb*B + k0
                        vcands = [("dve", DVE_TS), ("act", ACT_TS)]
                        if use_gps:
                            vcands.append(("gps", GPS_TS))
                        veng = pick(vcands)
                        if veng == "act":
                            nc.scalar.activation(v3, B_ap, COPY, bias=k0, scale=kb)
                        elif veng == "gps":
                            nc.gpsimd.tensor_scalar(v3, B_ap, kb, k0, MULT, ADD)
                        else:
                            nc.vector.tensor_scalar(v3, B_ap, kb, k0, MULT, ADD)
                    # w = u * A
                    wcands = [("dve", DVE_TT)]
                    if use_gps and not CFG.get("w_dve_only"):
                        wcands.append(("gps", GPS_TT))
                    weng = pick(wcands)
                    if weng == "gps":
                        nc.gpsimd.tensor_tensor(w3, u3, A_ap, MULT)
                    else:
                        nc.vector.tensor_tensor(w3, u3, A_ap, MULT)
                    # y = w + v, written p-interleaved into yc
                    yap = bass.AP(
                        ybase.tensor, ybase.offset + p4,
                        [[ypitch, P], [W * KPAIRS, OYS], [KPAIRS, W]],
                    )
                    ycands = [("dve", DVE_TT)]
                    if use_gps:
                        ycands.append(("gps", GPS_TT))
                    yeng = pick(ycands)
                    if not CFG.get("any_uv") and ueng == "pe":
                        # y = kab*w + v (kab deferred from the PE u form)
                        if yeng == "gps":
                            nc.gpsimd.scalar_tensor_tensor(yap, w3, kab, v3, MULT, ADD)
                        else:
                            nc.vector.scalar_tensor_tensor(yap, w3, kab, v3, MULT, ADD)
                    elif yeng == "gps":
                        nc.gpsimd.tensor_tensor(yap, w3, v3, ADD)
                    else:
                        nc.vector.tensor_tensor(yap, w3, v3, ADD)

                # DMA this channel out: HBM [n, oyblk, (oy',ox,p)=1024]
                oap = bass.AP(
                    out_d, cl * OUT_CSTRIDE,
                    [[OUT_NSTRIDE, N], [OYS * W * KPAIRS, OYB], [1, OYS * W * KPAIRS]],
                )
                nc.sync.dma_start(oap, yc[:])
    nc.finalize()  # Bacc: splits >1-wait syncs into event semaphores
    return nc


def _prep_inputs(x, weights, selection):
    x = np.ascontiguousarray(np.asarray(x, dtype=np.float32))
    weights = np.asarray(weights, dtype=np.float32)
    selection = np.asarray(selection, dtype=np.int32)

    # coefficients: softmax over 16 logic ops folded into {1,a,b,ab} basis
    w64 = weights.astype(np.float64)
    e = np.exp(w64 - w64.max(axis=-1, keepdims=True))
    prob = e / e.sum(axis=-1, keepdims=True)
    coef = (prob @ OP_COEFFS).astype(np.float32)  # [C_OUT, 4, 4]

    ch = ((selection >> 16) & 0xFFFF).astype(np.int64)
    ry = ((selection >> 8) & 0xFF).astype(np.int64)
    rx = (selection & 0xFF).astype(np.int64)

    # halo layout: xh[q=(n,oyblk), ch, r, w] = xpad[n, ch, oyblk*8+r, w]
    xpad = np.zeros((N, C_IN, H + 2, W + 2), dtype=np.float32)
    xpad[:, :, 1 : H + 1, 1 : W + 1] = x
    xh = np.empty((N, OYB, C_IN, HALO, W34), dtype=np.float32)
    for b in range(OYB):
        xh[:, b] = xpad[:, :, b * OYS : b * OYS + HALO, :]
    xh = np.ascontiguousarray(xh.reshape(P, XFREE))
    return xh, ch, ry, rx, coef


def kernel(x, weights, selection):
    assert x.shape == (N, C_IN, H, W), x.shape
    assert weights.shape == (C_OUT, 4, 16), weights.shape
    assert selection.shape == (C_OUT, 8), selection.shape

    xh, ch, ry, rx, coef = _prep_inputs(x, weights, selection)

    # per-core ka/kab bias rows for the TensorE u-path (0 where unused)
    kab_min = CFG.get("kab_min", 1e-3)
    kap_arrs = []
    for k in range(N_CORES):
        kap = np.zeros((P, CPC * KPAIRS), dtype=np.float32)
        for cl in range(CPC):
            c = k * CPC + cl
            for p4 in range(KPAIRS):
                kab_v = float(coef[c, p4, 3])
                if abs(kab_v) >= kab_min:
                    kap[:, cl * KPAIRS + p4] = float(coef[c, p4, 1]) / kab_v
        kap_arrs.append(kap)

    # Per-core auto-tune: each core's selection pattern favors a different
    # chunk1 size for the load/compute overlap — build a few candidates and
    # keep the one the TimelineSim cost model scores fastest.
    try:
        from concourse.timeline_sim import TimelineSim
    except Exception:  # noqa: BLE001
        TimelineSim = None
    cands = CFG.get("chunk_candidates", (32, 36, 40, 44))
    progs = []
    base_chunk = CFG["load_chunk_ch"]
    for k in range(N_CORES):
        best = None
        for cc in cands if TimelineSim is not None else (base_chunk,):
            CFG["load_chunk_ch"] = cc
            nc = build_core_program(k, ch, ry, rx, coef)
            ns = None
            if TimelineSim is not None:
                try:
                    ns = TimelineSim(nc, trace=False).simulate()
                except Exception:  # noqa: BLE001
                    ns = None
            if best is None or (ns is not None and best[0] is not None and ns < best[0]):
                best = (ns, nc)
            if ns is None:
                break
        progs.append(best[1])
        last_model_ns[k] = best[0]
    CFG["load_chunk_ch"] = base_chunk

    import jax

    devices = jax.devices()
    assert len(devices) >= N_CORES, devices

    outs = [None] * N_CORES
    errs = [None] * N_CORES
    # NTFF tracing needs axon hooks that aren't present in this container —
    # make sure run_bass_kernel_spmd never tries (BASS_TRACE in env would).
    os.environ["BASS_NEVER_TRACE"] = "1"

    def run_one(k):
        try:
            with jax.default_device(devices[k]):
                res = bass_utils.run_bass_kernel_spmd(
                    progs[k], [{"xh": xh, "kap": kap_arrs[k]}], core_ids=[k]
                )
            last_results[k] = res
            outs[k] = res.results[0]["out"]
        except Exception as e:  # noqa: BLE001
            errs[k] = e

    threads = [threading.Thread(target=run_one, args=(k,)) for k in range(N_CORES)]
    for t in threads:
        t.start()
    for t in threads:
        t.join()
    for k, e in enumerate(errs):
        if e is not None:
            raise RuntimeError(f"core {k} failed") from e

    y = np.empty((N, C_OUT, H, W, KPAIRS), dtype=np.float32)
    for k in range(N_CORES):
        y[:, k * CPC : (k + 1) * CPC] = outs[k]
    return y


# revision 39
# speedup vs baseline: 1.0051x; 1.0051x over previous
"""Trainium2 Bass kernel for nn_ConvLogicLayer.

Computes y[n,c,oy,ox,p] = k0 + ka*A + kb*B + kab*A*B where A/B are
shifted-window gathers of input channels (per the packed `selection`),
and k* are per-(c,p) coefficients derived from softmax(weights) @ OP_COEFFS.

Strategy:
  - Shard C_out (512) across 8 cores -> 64 output channels per core.
  - Each core gets a specialized program: the gather indices and the
    coefficients are baked into the instruction stream (static access
    patterns + immediate scalars), so the kernel is pure streaming
    elementwise work with zero gather traffic.
  - SBUF layout: partition q = n*4 + oyblk (32 images x 4 row-blocks),
    free dim = all 64 input channels x 10 halo rows x 34 padded cols.
    A shifted 8x32 window for any (ch,ry,rx) is then a single static
    3D access pattern on one SBUF tile.
  - Per (c,p) pair: u = kab*B + ka (ScalarE), v = kb*B + k0 and
    y = w + v load-balanced across VectorE/ScalarE/GPSIMD, w = u*A
    (VectorE).  y is written p-interleaved so the per-channel output DMA
    (512KB) is 4KB-contiguous in HBM.  Input load is chunked and ordered
    by first use so compute overlaps the streaming load.
    Load/compute overlap: 2-chunk load (most-used input channels first),
    output channels ordered by ready-pair count with ready pairs emitted
    first, and the chunk split auto-tuned per core via TimelineSim.
    Cost-model estimate: 167.7us (slowest core); measured rel err on
    hardware vs the f32 reference: 3.4e-07.
"""

import os
import sys
import threading

import numpy as np

for _p in ("/opt/trn_rl_repo",):
    if _p not in sys.path and os.path.isdir(_p):
        sys.path.insert(0, _p)

import concourse.bass as bass
import concourse.bacc as bacc
import concourse.mybir as mybir
from concourse.tile import TileContext
from concourse.masks import make_identity
from concourse import bass_utils

# Problem constants (hardcoded per spec)
N, C_IN, H, W = 32, 64, 32, 32
C_OUT, KPAIRS = 512, 4
N_CORES = 8
CPC = C_OUT // N_CORES  # channels per core

P = 128          # partitions = (n=32) x (oyblk=4)
OYB = 4          # oy blocks per image
OYS = 8          # oy rows per block
HALO = 10        # rows stored per block (8 + 2 halo)
W34 = 34         # padded width
CHSZ = HALO * W34           # 340 elems per (q, channel)
XFREE = C_IN * CHSZ         # 21760 elems per partition
OUT_CSTRIDE = H * W * KPAIRS          # 4096
OUT_NSTRIDE = CPC * OUT_CSTRIDE       # 262144

OP_COEFFS = np.array([
    [0.0, 0.0, 0.0, 0.0], [0.0, 0.0, 0.0, 1.0], [0.0, 1.0, 0.0, -1.0],
    [0.0, 1.0, 0.0, 0.0], [0.0, 0.0, 1.0, -1.0], [0.0, 0.0, 1.0, 0.0],
    [0.0, 1.0, 1.0, -2.0], [0.0, 1.0, 1.0, -1.0], [1.0, -1.0, -1.0, 1.0],
    [1.0, -1.0, -1.0, 2.0], [1.0, 0.0, -1.0, 0.0], [1.0, 0.0, -1.0, 1.0],
    [1.0, -1.0, 0.0, 0.0], [1.0, -1.0, 0.0, 1.0], [1.0, 0.0, 0.0, -1.0],
    [1.0, 0.0, 0.0, 0.0],
], dtype=np.float64)

MULT = mybir.AluOpType.mult
ADD = mybir.AluOpType.add
COPY = mybir.ActivationFunctionType.Copy

# Cost-model ns for load balancing (f32, [128, 256] tiles)
DVE_TT = 327.0   # tensor_tensor, 1x
DVE_TS = 194.0   # tensor_scalar, 2x_2P
ACT_TS = 507.0   # activation, 1x + 352cyc overhead
GPS_TS = 600.0   # gpsimd tensor_scalar (sw impl efficiency ~0.6)
GPS_TT = 600.0   # gpsimd tensor_tensor (sw impl efficiency ~0.42)
PE_U = 852.0     # two f32 matmuls (identity copy + bias row) on TensorE

# Tuning knobs (A/B'd via TimelineSim; best found = ~168us slowest core)
CFG = {
    "use_gps": True,     # offload v/y ops to GPSIMD
    "tp_bufs": 6,
    "yc_bufs": 8,
    "u_act_only": True,  # u always on ScalarE
    "w_dve_only": True,  # w always on VectorE
    "load_chunk_ch": 32,  # 2-chunk load (top-used channels first)
    "load_cascade": None,
    "any_uv": False,
    # TensorE u-offload (identity matmul + bias row, u = B + ka/kab, kab
    # re-applied at the y STT): numerically exact but model-NEGATIVE -- f32
    # matmul runs at 4 cycles/row plus cold p-state, and the PE->PSUM->DVE
    # chain serializes; every tested fraction lost ~7us. Kept for reference.
    "u_pe": False,
    "kab_min": 1e-3,     # |kab| guard for the u_pe refactoring
}

last_results = [None] * N_CORES  # BassKernelResults per core (for profiling)
last_model_ns = [None] * N_CORES  # per-core TimelineSim estimate of the shipped program


def build_core_program(core, ch, ry, rx, coef):
    """One specialized Bass program for `core` (channels core*CPC..+CPC)."""
    nc = bacc.Bacc("TRN2", target_bir_lowering=False)
    xh_d = nc.dram_tensor("xh", [P, XFREE], mybir.dt.float32, kind="ExternalInput")
    kap_d = nc.dram_tensor(
        "kap", [P, CPC * KPAIRS], mybir.dt.float32, kind="ExternalInput"
    )
    out_d = nc.dram_tensor(
        "out", [N, CPC, H, W, KPAIRS], mybir.dt.float32, kind="ExternalOutput"
    )

    use_gps = CFG["use_gps"]
    use_pe = CFG.get("u_pe")
    kab_min = CFG.get("kab_min", 1e-3)
    eng_ns = {"dve": 0.0, "act": 0.0, "gps": 0.0, "pe": 0.0}

    with TileContext(nc) as tc:
        with (
            tc.tile_pool(name="xp", bufs=1) as xpool,
            tc.tile_pool(name="tp", bufs=CFG["tp_bufs"]) as tpool,
            tc.tile_pool(name="yp", bufs=CFG["yc_bufs"]) as ypool,
            tc.tile_pool(name="pp", bufs=4, space="PSUM") as ppool,
        ):
            xh = xpool.tile([P, XFREE], mybir.dt.float32)
            # Jointly order output-channel processing (greedy: next output
            # needing fewest not-yet-loaded inputs) and stream input-channel
            # loads in that discovery order, so compute starts after a couple
            # of small chunks and fully overlaps the rest of the load.
            # (Tile's subtile tracking scopes each pair's waits to the load
            # DMAs it actually reads; Bacc splits any multi-wait syncs.)
            # Two-chunk load: chunk1 = the 32 most-used input channels, then
            # the rest.  A pair only needs its 2 input channels, so ~25-35%
            # of pairs are ready after chunk1 (~16us in); channels are ordered
            # by ready-pair count and ready pairs emitted first, so compute
            # overlaps the chunk2 load.  (Tile's subtile tracking scopes each
            # pair's waits to the load DMAs it reads.)
            chunk_ch = CFG.get("load_chunk_ch", 0)
            pair_chs = {
                (cl, p4): (int(ch[core * CPC + cl, 2 * p4]), int(ch[core * CPC + cl, 2 * p4 + 1]))
                for cl in range(CPC)
                for p4 in range(KPAIRS)
            }
            if chunk_ch <= 0:
                cl_order = list(range(CPC))
                p4_order = {cl: list(range(KPAIRS)) for cl in range(CPC)}
                nc.sync.dma_start(xh[:], xh_d[:])
            else:
                use_cnt = [0] * C_IN
                for a, b in pair_chs.values():
                    use_cnt[a] += 1
                    use_cnt[b] += 1
                by_use = sorted(range(C_IN), key=lambda i: -use_cnt[i])
                sizes = CFG.get("load_cascade") or [chunk_ch, C_IN - chunk_ch]
                tier_of = {}
                pos = 0
                groups = []
                for t, sz in enumerate(sizes):
                    grp = by_use[pos : pos + sz]
                    pos += sz
                    for cch in grp:
                        tier_of[cch] = t
                    if grp:
                        groups.append(sorted(grp))
                ptier = {
                    (cl, p4): max(tier_of[a], tier_of[b])
                    for (cl, p4), (a, b) in pair_chs.items()
                }
                tiers = {cl: sorted(ptier[(cl, p4)] for p4 in range(KPAIRS)) for cl in range(CPC)}
                cl_order = sorted(range(CPC), key=lambda cl: tiers[cl])
                p4_order = {
                    cl: sorted(range(KPAIRS), key=lambda p4: ptier[(cl, p4)])
                    for cl in range(CPC)
                }
                for grp in groups:
                    run = [grp[0]]
                    for cch in grp[1:] + [None]:
                        if cch is not None and cch == run[-1] + 1:
                            run.append(cch)
                            continue
                        lo, hi = run[0] * CHSZ, (run[-1] + 1) * CHSZ
                        nc.sync.dma_start(xh[:, lo:hi], xh_d[:, lo:hi])
                        if cch is not None:
                            run = [cch]
            base = xh[:]
            pitch = base.ap[0][0]
            tens = base.tensor
            base_off = base.offset

            if use_pe:
                ident = xpool.tile([P, P], mybir.dt.float32, tag="ident")
                ones = xpool.tile([1, W * OYS], mybir.dt.float32, tag="ones")
                kap = xpool.tile([P, CPC * KPAIRS], mybir.dt.float32, tag="kap")
                make_identity(nc, ident[:])
                nc.vector.memset(ones[:], 1.0)
                nc.sync.dma_start(kap[:], kap_d[:])

            for cl in cl_order:
                c = core * CPC + cl
                yc = ypool.tile([P, OYS * W * KPAIRS], mybir.dt.float32, tag="yc")
                ybase = yc[:]
                ypitch = ybase.ap[0][0]
                for p4 in p4_order[cl]:
                    ka_, kb_ = 2 * p4, 2 * p4 + 1
                    offA = base_off + int(ch[c, ka_]) * CHSZ + int(ry[c, ka_]) * W34 + int(rx[c, ka_])
                    offB = base_off + int(ch[c, kb_]) * CHSZ + int(ry[c, kb_]) * W34 + int(rx[c, kb_])
                    A_ap = bass.AP(tens, offA, [[pitch, P], [W34, OYS], [1, W]])
                    B_ap = bass.AP(tens, offB, [[pitch, P], [W34, OYS], [1, W]])

                    k0 = float(coef[c, p4, 0])
                    ka = float(coef[c, p4, 1])
                    kb = float(coef[c, p4, 2])
                    kab = float(coef[c, p4, 3])

                    u = tpool.tile([P, OYS * W], mybir.dt.float32, tag="u")
                    v = tpool.tile([P, OYS * W], mybir.dt.float32, tag="v")
                    w = tpool.tile([P, OYS * W], mybir.dt.float32, tag="w")
                    u3 = u[:].rearrange("p (a b) -> p a b", b=W)
                    v3 = v[:].rearrange("p (a b) -> p a b", b=W)
                    w3 = w[:].rearrange("p (a b) -> p a b", b=W)

                    def pick(cands):
                        eng, cost = min(cands, key=lambda c: eng_ns[c[0]] + c[1])
                        eng_ns[eng] += cost
                        return eng

                    # u = kab*B + ka
                    pe_ok = use_pe and abs(kab) >= kab_min
                    ueng = None
                    if CFG.get("any_uv"):
                        nc.any.tensor_scalar(u3, B_ap, kab, ka, MULT, ADD)
                        nc.any.tensor_scalar(v3, B_ap, kb, k0, MULT, ADD)
                        eng_ns["act"] += ACT_TS  # rough accounting
                        eng_ns["dve"] += DVE_TS
                    else:
                        if CFG.get("u_act_only"):
                            ucands = [("act", ACT_TS)]
                        else:
                            ucands = [("act", ACT_TS), ("dve", DVE_TS)]
                            if use_gps:
                                ucands.append(("gps", GPS_TS))
                        if pe_ok:
                            ucands = ucands + [("pe", PE_U)]
                        ueng = pick(ucands)
                        if ueng == "pe":
                            # u = B + ka/kab via identity matmul + bias row;
                            # kab is re-applied at the y step (STT below).
                            upsum = ppool.tile([P, OYS * W], mybir.dt.float32, tag="up")
                            j = cl * KPAIRS + p4
                            nc.tensor.matmul(
                                out=upsum[:], lhsT=ident[:], rhs=B_ap,
                                start=True, stop=False,
                            )
                            nc.tensor.matmul(
                                out=upsum[:],
                                lhsT=kap[0:1, j : j + 1].to_broadcast((1, P)),
                                rhs=ones[:], start=False, stop=True,
                            )
                            u3 = upsum[:].rearrange("p (a b) -> p a b", b=W)
                        elif ueng == "act":
                            nc.scalar.activation(u3, B_ap, COPY, bias=ka, scale=kab)
                        elif ueng == "gps":
                            nc.gpsimd.tensor_scalar(u3, B_ap, kab, ka, MULT, ADD)
                        else:
                            nc.vector.tensor_scalar(u3, B_ap, kab, ka, MULT, ADD)
                        # v = kb*B + k0
                        vcands = [("dve", DVE_TS), ("act", ACT_TS)]
                        if use_gps:
                            vcands.append(("gps", GPS_TS))
                        veng = pick(vcands)
                        if veng == "act":
                            nc.scalar.activation(v3, B_ap, COPY, bias=k0, scale=kb)
                        elif veng == "gps":
                            nc.gpsimd.tensor_scalar(v3, B_ap, kb, k0, MULT, ADD)
                        else:
                            nc.vector.tensor_scalar(v3, B_ap, kb, k0, MULT, ADD)
                    # w = u * A
                    wcands = [("dve", DVE_TT)]
                    if use_gps and not CFG.get("w_dve_only"):
                        wcands.append(("gps", GPS_TT))
                    weng = pick(wcands)
                    if weng == "gps":
                        nc.gpsimd.tensor_tensor(w3, u3, A_ap, MULT)
                    else:
                        nc.vector.tensor_tensor(w3, u3, A_ap, MULT)
                    # y = w + v, written p-interleaved into yc
                    yap = bass.AP(
                        ybase.tensor, ybase.offset + p4,
                        [[ypitch, P], [W * KPAIRS, OYS], [KPAIRS, W]],
                    )
                    ycands = [("dve", DVE_TT)]
                    if use_gps:
                        ycands.append(("gps", GPS_TT))
                    yeng = pick(ycands)
                    if not CFG.get("any_uv") and ueng == "pe":
                        # y = kab*w + v (kab deferred from the PE u form)
                        if yeng == "gps":
                            nc.gpsimd.scalar_tensor_tensor(yap, w3, kab, v3, MULT, ADD)
                        else:
                            nc.vector.scalar_tensor_tensor(yap, w3, kab, v3, MULT, ADD)
                    elif yeng == "gps":
                        nc.gpsimd.tensor_tensor(yap, w3, v3, ADD)
                    else:
                        nc.vector.tensor_tensor(yap, w3, v3, ADD)

                # DMA this channel out: HBM [n, oyblk, (oy',ox,p)=1024]
                oap = bass.AP(
                    out_d, cl * OUT_CSTRIDE,
                    [[OUT_NSTRIDE, N], [OYS * W * KPAIRS, OYB], [1, OYS * W * KPAIRS]],
                )
                nc.sync.dma_start(oap, yc[:])
    nc.finalize()  # Bacc: splits >1-wait syncs into event semaphores
    return nc


def _prep_inputs(x, weights, selection):
    x = np.ascontiguousarray(np.asarray(x, dtype=np.float32))
    weights = np.asarray(weights, dtype=np.float32)
    selection = np.asarray(selection, dtype=np.int32)

    # coefficients: softmax over 16 logic ops folded into {1,a,b,ab} basis
    w64 = weights.astype(np.float64)
    e = np.exp(w64 - w64.max(axis=-1, keepdims=True))
    prob = e / e.sum(axis=-1, keepdims=True)
    coef = (prob @ OP_COEFFS).astype(np.float32)  # [C_OUT, 4, 4]

    ch = ((selection >> 16) & 0xFFFF).astype(np.int64)
    ry = ((selection >> 8) & 0xFF).astype(np.int64)
    rx = (selection & 0xFF).astype(np.int64)

    # halo layout: xh[q=(n,oyblk), ch, r, w] = xpad[n, ch, oyblk*8+r, w]
    xpad = np.zeros((N, C_IN, H + 2, W + 2), dtype=np.float32)
    xpad[:, :, 1 : H + 1, 1 : W + 1] = x
    xh = np.empty((N, OYB, C_IN, HALO, W34), dtype=np.float32)
    for b in range(OYB):
        xh[:, b] = xpad[:, :, b * OYS : b * OYS + HALO, :]
    xh = np.ascontiguousarray(xh.reshape(P, XFREE))
    return xh, ch, ry, rx, coef


def kernel(x, weights, selection):
    assert x.shape == (N, C_IN, H, W), x.shape
    assert weights.shape == (C_OUT, 4, 16), weights.shape
    assert selection.shape == (C_OUT, 8), selection.shape

    xh, ch, ry, rx, coef = _prep_inputs(x, weights, selection)

    # per-core ka/kab bias rows for the TensorE u-path (0 where unused)
    kab_min = CFG.get("kab_min", 1e-3)
    kap_arrs = []
    for k in range(N_CORES):
        kap = np.zeros((P, CPC * KPAIRS), dtype=np.float32)
        for cl in range(CPC):
            c = k * CPC + cl
            for p4 in range(KPAIRS):
                kab_v = float(coef[c, p4, 3])
                if abs(kab_v) >= kab_min:
                    kap[:, cl * KPAIRS + p4] = float(coef[c, p4, 1]) / kab_v
        kap_arrs.append(kap)

    # Per-core auto-tune: each core's selection pattern favors a different
    # chunk1 size for the load/compute overlap — build a few candidates and
    # keep the one the TimelineSim cost model scores fastest.
    try:
        from concourse.timeline_sim import TimelineSim
    except Exception:  # noqa: BLE001
        TimelineSim = None
    cands = CFG.get(
        "tune_candidates",
        ((32, 507.0), (36, 507.0), (40, 507.0), (44, 507.0), (36, 480.0), (40, 480.0)),
    )
    progs = []
    base_chunk = CFG["load_chunk_ch"]
    global ACT_TS
    base_act = ACT_TS
    for k in range(N_CORES):
        best = None
        for cc, act in cands if TimelineSim is not None else ((base_chunk, base_act),):
            CFG["load_chunk_ch"] = cc
            ACT_TS = act
            nc = build_core_program(k, ch, ry, rx, coef)
            ns = None
            if TimelineSim is not None:
                try:
                    ns = TimelineSim(nc, trace=False).simulate()
                except Exception:  # noqa: BLE001
                    ns = None
            if best is None or (ns is not None and best[0] is not None and ns < best[0]):
                best = (ns, nc)
            if ns is None:
                break
        progs.append(best[1])
        last_model_ns[k] = best[0]
    CFG["load_chunk_ch"] = base_chunk
    ACT_TS = base_act

    import jax

    devices = jax.devices()
    assert len(devices) >= N_CORES, devices

    outs = [None] * N_CORES
    errs = [None] * N_CORES
    # NTFF tracing needs axon hooks that aren't present in this container —
    # make sure run_bass_kernel_spmd never tries (BASS_TRACE in env would).
    os.environ["BASS_NEVER_TRACE"] = "1"

    def run_one(k):
        try:
            with jax.default_device(devices[k]):
                res = bass_utils.run_bass_kernel_spmd(
                    progs[k], [{"xh": xh, "kap": kap_arrs[k]}], core_ids=[k]
                )
            last_results[k] = res
            outs[k] = res.results[0]["out"]
        except Exception as e:  # noqa: BLE001
            errs[k] = e

    threads = [threading.Thread(target=run_one, args=(k,)) for k in range(N_CORES)]
    for t in threads:
        t.start()
    for t in threads:
        t.join()
    for k, e in enumerate(errs):
        if e is not None:
            raise RuntimeError(f"core {k} failed") from e

    y = np.empty((N, C_OUT, H, W, KPAIRS), dtype=np.float32)
    for k in range(N_CORES):
        y[:, k * CPC : (k + 1) * CPC] = outs[k]
    return y


# revision 40
# speedup vs baseline: 1.0104x; 1.0052x over previous
"""Trainium2 Bass kernel for nn_ConvLogicLayer.

Computes y[n,c,oy,ox,p] = k0 + ka*A + kb*B + kab*A*B where A/B are
shifted-window gathers of input channels (per the packed `selection`),
and k* are per-(c,p) coefficients derived from softmax(weights) @ OP_COEFFS.

Strategy:
  - Shard C_out (512) across 8 cores -> 64 output channels per core.
  - Each core gets a specialized program: the gather indices and the
    coefficients are baked into the instruction stream (static access
    patterns + immediate scalars), so the kernel is pure streaming
    elementwise work with zero gather traffic.
  - SBUF layout: partition q = n*4 + oyblk (32 images x 4 row-blocks),
    free dim = all 64 input channels x 10 halo rows x 34 padded cols.
    A shifted 8x32 window for any (ch,ry,rx) is then a single static
    3D access pattern on one SBUF tile.
  - Per (c,p) pair: u = kab*B + ka (ScalarE), v = kb*B + k0 and
    y = w + v load-balanced across VectorE/ScalarE/GPSIMD, w = u*A
    (VectorE).  y is written p-interleaved so the per-channel output DMA
    (512KB) is 4KB-contiguous in HBM.  Input load is chunked and ordered
    by first use so compute overlaps the streaming load.
    Load/compute overlap: 2-chunk load (most-used input channels first),
    output channels ordered by ready-pair count with ready pairs emitted
    first, and the chunk split auto-tuned per core via TimelineSim.
    Cost-model estimate: 167.7us (slowest core); measured rel err on
    hardware vs the f32 reference: 3.4e-07.
"""

import os
import sys
import threading

import numpy as np

for _p in ("/opt/trn_rl_repo",):
    if _p not in sys.path and os.path.isdir(_p):
        sys.path.insert(0, _p)

import concourse.bass as bass
import concourse.bacc as bacc
import concourse.mybir as mybir
from concourse.tile import TileContext
from concourse.masks import make_identity
from concourse import bass_utils

# Problem constants (hardcoded per spec)
N, C_IN, H, W = 32, 64, 32, 32
C_OUT, KPAIRS = 512, 4
N_CORES = 8
CPC = C_OUT // N_CORES  # channels per core

P = 128          # partitions = (n=32) x (oyblk=4)
OYB = 4          # oy blocks per image
OYS = 8          # oy rows per block
HALO = 10        # rows stored per block (8 + 2 halo)
W34 = 34         # padded width
CHSZ = HALO * W34           # 340 elems per (q, channel)
XFREE = C_IN * CHSZ         # 21760 elems per partition
OUT_CSTRIDE = H * W * KPAIRS          # 4096
OUT_NSTRIDE = CPC * OUT_CSTRIDE       # 262144

OP_COEFFS = np.array([
    [0.0, 0.0, 0.0, 0.0], [0.0, 0.0, 0.0, 1.0], [0.0, 1.0, 0.0, -1.0],
    [0.0, 1.0, 0.0, 0.0], [0.0, 0.0, 1.0, -1.0], [0.0, 0.0, 1.0, 0.0],
    [0.0, 1.0, 1.0, -2.0], [0.0, 1.0, 1.0, -1.0], [1.0, -1.0, -1.0, 1.0],
    [1.0, -1.0, -1.0, 2.0], [1.0, 0.0, -1.0, 0.0], [1.0, 0.0, -1.0, 1.0],
    [1.0, -1.0, 0.0, 0.0], [1.0, -1.0, 0.0, 1.0], [1.0, 0.0, 0.0, -1.0],
    [1.0, 0.0, 0.0, 0.0],
], dtype=np.float64)

MULT = mybir.AluOpType.mult
ADD = mybir.AluOpType.add
COPY = mybir.ActivationFunctionType.Copy

# Cost-model ns for load balancing (f32, [128, 256] tiles)
DVE_TT = 327.0   # tensor_tensor, 1x
DVE_TS = 194.0   # tensor_scalar, 2x_2P
ACT_TS = 507.0   # activation, 1x + 352cyc overhead
GPS_TS = 600.0   # gpsimd tensor_scalar (sw impl efficiency ~0.6)
GPS_TT = 600.0   # gpsimd tensor_tensor (sw impl efficiency ~0.42)
PE_U = 852.0     # two f32 matmuls (identity copy + bias row) on TensorE

# Tuning knobs (A/B'd via TimelineSim; best found = ~168us slowest core)
CFG = {
    "use_gps": True,     # offload v/y ops to GPSIMD
    "tp_bufs": 6,
    "yc_bufs": 8,
    "u_act_only": True,  # u always on ScalarE
    "w_dve_only": True,  # w always on VectorE
    "load_chunk_ch": 32,  # 2-chunk load (top-used channels first)
    "load_cascade": None,
    "any_uv": False,
    # TensorE u-offload (identity matmul + bias row, u = B + ka/kab, kab
    # re-applied at the y STT): numerically exact but model-NEGATIVE -- f32
    # matmul runs at 4 cycles/row plus cold p-state, and the PE->PSUM->DVE
    # chain serializes; every tested fraction lost ~7us. Kept for reference.
    "u_pe": False,
    "kab_min": 1e-3,     # |kab| guard for the u_pe refactoring
}

last_results = [None] * N_CORES  # BassKernelResults per core (for profiling)
last_model_ns = [None] * N_CORES  # per-core TimelineSim estimate of the shipped program


def build_core_program(core, ch, ry, rx, coef):
    """One specialized Bass program for `core` (channels core*CPC..+CPC)."""
    nc = bacc.Bacc("TRN2", target_bir_lowering=False)
    xh_d = nc.dram_tensor("xh", [P, XFREE], mybir.dt.float32, kind="ExternalInput")
    kap_d = nc.dram_tensor(
        "kap", [P, CPC * KPAIRS], mybir.dt.float32, kind="ExternalInput"
    )
    out_d = nc.dram_tensor(
        "out", [N, CPC, H, W, KPAIRS], mybir.dt.float32, kind="ExternalOutput"
    )

    use_gps = CFG["use_gps"]
    use_pe = CFG.get("u_pe")
    kab_min = CFG.get("kab_min", 1e-3)
    eng_ns = {"dve": 0.0, "act": 0.0, "gps": 0.0, "pe": 0.0}

    with TileContext(nc) as tc:
        with (
            tc.tile_pool(name="xp", bufs=1) as xpool,
            tc.tile_pool(name="tp", bufs=CFG["tp_bufs"]) as tpool,
            tc.tile_pool(name="yp", bufs=CFG["yc_bufs"]) as ypool,
            tc.tile_pool(name="pp", bufs=4, space="PSUM") as ppool,
        ):
            xh = xpool.tile([P, XFREE], mybir.dt.float32)
            # Jointly order output-channel processing (greedy: next output
            # needing fewest not-yet-loaded inputs) and stream input-channel
            # loads in that discovery order, so compute starts after a couple
            # of small chunks and fully overlaps the rest of the load.
            # (Tile's subtile tracking scopes each pair's waits to the load
            # DMAs it actually reads; Bacc splits any multi-wait syncs.)
            # Two-chunk load: chunk1 = the 32 most-used input channels, then
            # the rest.  A pair only needs its 2 input channels, so ~25-35%
            # of pairs are ready after chunk1 (~16us in); channels are ordered
            # by ready-pair count and ready pairs emitted first, so compute
            # overlaps the chunk2 load.  (Tile's subtile tracking scopes each
            # pair's waits to the load DMAs it reads.)
            chunk_ch = CFG.get("load_chunk_ch", 0)
            pair_chs = {
                (cl, p4): (int(ch[core * CPC + cl, 2 * p4]), int(ch[core * CPC + cl, 2 * p4 + 1]))
                for cl in range(CPC)
                for p4 in range(KPAIRS)
            }
            if chunk_ch <= 0:
                cl_order = list(range(CPC))
                p4_order = {cl: list(range(KPAIRS)) for cl in range(CPC)}
                nc.sync.dma_start(xh[:], xh_d[:])
            else:
                use_cnt = [0] * C_IN
                for a, b in pair_chs.values():
                    use_cnt[a] += 1
                    use_cnt[b] += 1
                by_use = sorted(range(C_IN), key=lambda i: -use_cnt[i])
                sizes = CFG.get("load_cascade") or [chunk_ch, C_IN - chunk_ch]
                tier_of = {}
                pos = 0
                groups = []
                for t, sz in enumerate(sizes):
                    grp = by_use[pos : pos + sz]
                    pos += sz
                    for cch in grp:
                        tier_of[cch] = t
                    if grp:
                        groups.append(sorted(grp))
                ptier = {
                    (cl, p4): max(tier_of[a], tier_of[b])
                    for (cl, p4), (a, b) in pair_chs.items()
                }
                tiers = {cl: sorted(ptier[(cl, p4)] for p4 in range(KPAIRS)) for cl in range(CPC)}
                cl_order = sorted(range(CPC), key=lambda cl: tiers[cl])
                p4_order = {
                    cl: sorted(range(KPAIRS), key=lambda p4: ptier[(cl, p4)])
                    for cl in range(CPC)
                }
                for grp in groups:
                    run = [grp[0]]
                    for cch in grp[1:] + [None]:
                        if cch is not None and cch == run[-1] + 1:
                            run.append(cch)
                            continue
                        lo, hi = run[0] * CHSZ, (run[-1] + 1) * CHSZ
                        nc.sync.dma_start(xh[:, lo:hi], xh_d[:, lo:hi])
                        if cch is not None:
                            run = [cch]
            base = xh[:]
            pitch = base.ap[0][0]
            tens = base.tensor
            base_off = base.offset

            if use_pe:
                ident = xpool.tile([P, P], mybir.dt.float32, tag="ident")
                ones = xpool.tile([1, W * OYS], mybir.dt.float32, tag="ones")
                kap = xpool.tile([P, CPC * KPAIRS], mybir.dt.float32, tag="kap")
                make_identity(nc, ident[:])
                nc.vector.memset(ones[:], 1.0)
                nc.sync.dma_start(kap[:], kap_d[:])

            for cl in cl_order:
                c = core * CPC + cl
                yc = ypool.tile([P, OYS * W * KPAIRS], mybir.dt.float32, tag="yc")
                ybase = yc[:]
                ypitch = ybase.ap[0][0]
                for p4 in p4_order[cl]:
                    ka_, kb_ = 2 * p4, 2 * p4 + 1
                    offA = base_off + int(ch[c, ka_]) * CHSZ + int(ry[c, ka_]) * W34 + int(rx[c, ka_])
                    offB = base_off + int(ch[c, kb_]) * CHSZ + int(ry[c, kb_]) * W34 + int(rx[c, kb_])
                    A_ap = bass.AP(tens, offA, [[pitch, P], [W34, OYS], [1, W]])
                    B_ap = bass.AP(tens, offB, [[pitch, P], [W34, OYS], [1, W]])

                    k0 = float(coef[c, p4, 0])
                    ka = float(coef[c, p4, 1])
                    kb = float(coef[c, p4, 2])
                    kab = float(coef[c, p4, 3])

                    u = tpool.tile([P, OYS * W], mybir.dt.float32, tag="u")
                    v = tpool.tile([P, OYS * W], mybir.dt.float32, tag="v")
                    w = tpool.tile([P, OYS * W], mybir.dt.float32, tag="w")
                    u3 = u[:].rearrange("p (a b) -> p a b", b=W)
                    v3 = v[:].rearrange("p (a b) -> p a b", b=W)
                    w3 = w[:].rearrange("p (a b) -> p a b", b=W)

                    def pick(cands):
                        eng, cost = min(cands, key=lambda c: eng_ns[c[0]] + c[1])
                        eng_ns[eng] += cost
                        return eng

                    # u = kab*B + ka
                    pe_ok = use_pe and abs(kab) >= kab_min
                    ueng = None
                    if CFG.get("any_uv"):
                        nc.any.tensor_scalar(u3, B_ap, kab, ka, MULT, ADD)
                        nc.any.tensor_scalar(v3, B_ap, kb, k0, MULT, ADD)
                        eng_ns["act"] += ACT_TS  # rough accounting
                        eng_ns["dve"] += DVE_TS
                    else:
                        if CFG.get("u_act_only"):
                            ucands = [("act", ACT_TS)]
                        else:
                            ucands = [("act", ACT_TS), ("dve", DVE_TS)]
                            if use_gps:
                                ucands.append(("gps", GPS_TS))
                        if pe_ok:
                            ucands = ucands + [("pe", PE_U)]
                        ueng = pick(ucands)
                        if ueng == "pe":
                            # u = B + ka/kab via identity matmul + bias row;
                            # kab is re-applied at the y step (STT below).
                            upsum = ppool.tile([P, OYS * W], mybir.dt.float32, tag="up")
                            j = cl * KPAIRS + p4
                            nc.tensor.matmul(
                                out=upsum[:], lhsT=ident[:], rhs=B_ap,
                                start=True, stop=False,
                            )
                            nc.tensor.matmul(
                                out=upsum[:],
                                lhsT=kap[0:1, j : j + 1].to_broadcast((1, P)),
                                rhs=ones[:], start=False, stop=True,
                            )
                            u3 = upsum[:].rearrange("p (a b) -> p a b", b=W)
                        elif ueng == "act":
                            nc.scalar.activation(u3, B_ap, COPY, bias=ka, scale=kab)
                        elif ueng == "gps":
                            nc.gpsimd.tensor_scalar(u3, B_ap, kab, ka, MULT, ADD)
                        else:
                            nc.vector.tensor_scalar(u3, B_ap, kab, ka, MULT, ADD)
                        # v = kb*B + k0
                        vcands = [("dve", DVE_TS), ("act", ACT_TS)]
                        if use_gps:
                            vcands.append(("gps", GPS_TS))
                        veng = pick(vcands)
                        if veng == "act":
                            nc.scalar.activation(v3, B_ap, COPY, bias=k0, scale=kb)
                        elif veng == "gps":
                            nc.gpsimd.tensor_scalar(v3, B_ap, kb, k0, MULT, ADD)
                        else:
                            nc.vector.tensor_scalar(v3, B_ap, kb, k0, MULT, ADD)
                    # w = u * A
                    wcands = [("dve", DVE_TT)]
                    if use_gps and not CFG.get("w_dve_only"):
                        wcands.append(("gps", GPS_TT))
                    weng = pick(wcands)
                    if weng == "gps":
                        nc.gpsimd.tensor_tensor(w3, u3, A_ap, MULT)
                    else:
                        nc.vector.tensor_tensor(w3, u3, A_ap, MULT)
                    # y = w + v, written p-interleaved into yc
                    yap = bass.AP(
                        ybase.tensor, ybase.offset + p4,
                        [[ypitch, P], [W * KPAIRS, OYS], [KPAIRS, W]],
                    )
                    ycands = [("dve", DVE_TT)]
                    if use_gps:
                        ycands.append(("gps", GPS_TT))
                    yeng = pick(ycands)
                    if not CFG.get("any_uv") and ueng == "pe":
                        # y = kab*w + v (kab deferred from the PE u form)
                        if yeng == "gps":
                            nc.gpsimd.scalar_tensor_tensor(yap, w3, kab, v3, MULT, ADD)
                        else:
                            nc.vector.scalar_tensor_tensor(yap, w3, kab, v3, MULT, ADD)
                    elif yeng == "gps":
                        nc.gpsimd.tensor_tensor(yap, w3, v3, ADD)
                    else:
                        nc.vector.tensor_tensor(yap, w3, v3, ADD)

                # DMA this channel out: HBM [n, oyblk, (oy',ox,p)=1024]
                oap = bass.AP(
                    out_d, cl * OUT_CSTRIDE,
                    [[OUT_NSTRIDE, N], [OYS * W * KPAIRS, OYB], [1, OYS * W * KPAIRS]],
                )
                nc.sync.dma_start(oap, yc[:])
    nc.finalize()  # Bacc: splits >1-wait syncs into event semaphores
    return nc


def _prep_inputs(x, weights, selection):
    x = np.ascontiguousarray(np.asarray(x, dtype=np.float32))
    weights = np.asarray(weights, dtype=np.float32)
    selection = np.asarray(selection, dtype=np.int32)

    # coefficients: softmax over 16 logic ops folded into {1,a,b,ab} basis
    w64 = weights.astype(np.float64)
    e = np.exp(w64 - w64.max(axis=-1, keepdims=True))
    prob = e / e.sum(axis=-1, keepdims=True)
    coef = (prob @ OP_COEFFS).astype(np.float32)  # [C_OUT, 4, 4]

    ch = ((selection >> 16) & 0xFFFF).astype(np.int64)
    ry = ((selection >> 8) & 0xFF).astype(np.int64)
    rx = (selection & 0xFF).astype(np.int64)

    # halo layout: xh[q=(n,oyblk), ch, r, w] = xpad[n, ch, oyblk*8+r, w]
    xpad = np.zeros((N, C_IN, H + 2, W + 2), dtype=np.float32)
    xpad[:, :, 1 : H + 1, 1 : W + 1] = x
    xh = np.empty((N, OYB, C_IN, HALO, W34), dtype=np.float32)
    for b in range(OYB):
        xh[:, b] = xpad[:, :, b * OYS : b * OYS + HALO, :]
    xh = np.ascontiguousarray(xh.reshape(P, XFREE))
    return xh, ch, ry, rx, coef


def kernel(x, weights, selection):
    assert x.shape == (N, C_IN, H, W), x.shape
    assert weights.shape == (C_OUT, 4, 16), weights.shape
    assert selection.shape == (C_OUT, 8), selection.shape

    xh, ch, ry, rx, coef = _prep_inputs(x, weights, selection)

    # per-core ka/kab bias rows for the TensorE u-path (0 where unused)
    kab_min = CFG.get("kab_min", 1e-3)
    kap_arrs = []
    for k in range(N_CORES):
        kap = np.zeros((P, CPC * KPAIRS), dtype=np.float32)
        for cl in range(CPC):
            c = k * CPC + cl
            for p4 in range(KPAIRS):
                kab_v = float(coef[c, p4, 3])
                if abs(kab_v) >= kab_min:
                    kap[:, cl * KPAIRS + p4] = float(coef[c, p4, 1]) / kab_v
        kap_arrs.append(kap)

    # Per-core auto-tune: each core's selection pattern favors a different
    # chunk1 size for the load/compute overlap — build a few candidates and
    # keep the one the TimelineSim cost model scores fastest.
    try:
        from concourse.timeline_sim import TimelineSim
    except Exception:  # noqa: BLE001
        TimelineSim = None
    cands = CFG.get(
        "tune_candidates",
        (
            (32, 507.0, 600.0), (36, 507.0, 600.0), (40, 507.0, 600.0),
            (44, 507.0, 600.0), (36, 480.0, 600.0), (40, 480.0, 600.0),
            (40, 480.0, 500.0), (36, 480.0, 500.0),
        ),
    )
    progs = []
    base_chunk = CFG["load_chunk_ch"]
    global ACT_TS, GPS_TS
    base_act, base_gts = ACT_TS, GPS_TS
    for k in range(N_CORES):
        best = None
        for cc, act, gts in cands if TimelineSim is not None else ((base_chunk, base_act, base_gts),):
            CFG["load_chunk_ch"] = cc
            ACT_TS, GPS_TS = act, gts
            nc = build_core_program(k, ch, ry, rx, coef)
            ns = None
            if TimelineSim is not None:
                try:
                    ns = TimelineSim(nc, trace=False).simulate()
                except Exception:  # noqa: BLE001
                    ns = None
            if best is None or (ns is not None and best[0] is not None and ns < best[0]):
                best = (ns, nc)
            if ns is None:
                break
        progs.append(best[1])
        last_model_ns[k] = best[0]
    CFG["load_chunk_ch"] = base_chunk
    ACT_TS, GPS_TS = base_act, base_gts

    import jax

    devices = jax.devices()
    assert len(devices) >= N_CORES, devices

    outs = [None] * N_CORES
    errs = [None] * N_CORES
    # NTFF tracing needs axon hooks that aren't present in this container —
    # make sure run_bass_kernel_spmd never tries (BASS_TRACE in env would).
    os.environ["BASS_NEVER_TRACE"] = "1"

    def run_one(k):
        try:
            with jax.default_device(devices[k]):
                res = bass_utils.run_bass_kernel_spmd(
                    progs[k], [{"xh": xh, "kap": kap_arrs[k]}], core_ids=[k]
                )
            last_results[k] = res
            outs[k] = res.results[0]["out"]
        except Exception as e:  # noqa: BLE001
            errs[k] = e

    threads = [threading.Thread(target=run_one, args=(k,)) for k in range(N_CORES)]
    for t in threads:
        t.start()
    for t in threads:
        t.join()
    for k, e in enumerate(errs):
        if e is not None:
            raise RuntimeError(f"core {k} failed") from e

    y = np.empty((N, C_OUT, H, W, KPAIRS), dtype=np.float32)
    for k in range(N_CORES):
        y[:, k * CPC : (k + 1) * CPC] = outs[k]
    return y


# revision 42
# speedup vs baseline: 1.0153x; 1.0049x over previous
"""Trainium2 Bass kernel for nn_ConvLogicLayer.

Computes y[n,c,oy,ox,p] = k0 + ka*A + kb*B + kab*A*B where A/B are
shifted-window gathers of input channels (per the packed `selection`),
and k* are per-(c,p) coefficients derived from softmax(weights) @ OP_COEFFS.

Strategy:
  - Shard C_out (512) across 8 cores -> 64 output channels per core.
  - Each core gets a specialized program: the gather indices and the
    coefficients are baked into the instruction stream (static access
    patterns + immediate scalars), so the kernel is pure streaming
    elementwise work with zero gather traffic.
  - SBUF layout: partition q = n*4 + oyblk (32 images x 4 row-blocks),
    free dim = all 64 input channels x 10 halo rows x 34 padded cols.
    A shifted 8x32 window for any (ch,ry,rx) is then a single static
    3D access pattern on one SBUF tile.
  - Per (c,p) pair: u = kab*B + ka (ScalarE), v = kb*B + k0 and
    y = w + v load-balanced across VectorE/ScalarE/GPSIMD, w = u*A
    (VectorE).  y is written p-interleaved so the per-channel output DMA
    (512KB) is 4KB-contiguous in HBM.  Input load is chunked and ordered
    by first use so compute overlaps the streaming load.
    Load/compute overlap: 2-chunk load (most-used input channels first),
    output channels ordered by ready-pair count with ready pairs emitted
    first, and the chunk split auto-tuned per core via TimelineSim.
    Per-core auto-tune picks (load-chunk, greedy-balance constants) by
    TimelineSim.  Cost-model estimate: 166.0us (slowest core); measured
    rel err on hardware vs the f32 reference: 3.4e-07.
"""

import os
import sys
import threading

import numpy as np

for _p in ("/opt/trn_rl_repo",):
    if _p not in sys.path and os.path.isdir(_p):
        sys.path.insert(0, _p)

import concourse.bass as bass
import concourse.bacc as bacc
import concourse.mybir as mybir
from concourse.tile import TileContext
from concourse.masks import make_identity
from concourse import bass_utils

# Problem constants (hardcoded per spec)
N, C_IN, H, W = 32, 64, 32, 32
C_OUT, KPAIRS = 512, 4
N_CORES = 8
CPC = C_OUT // N_CORES  # channels per core

P = 128          # partitions = (n=32) x (oyblk=4)
OYB = 4          # oy blocks per image
OYS = 8          # oy rows per block
HALO = 10        # rows stored per block (8 + 2 halo)
W34 = 34         # padded width
CHSZ = HALO * W34           # 340 elems per (q, channel)
XFREE = C_IN * CHSZ         # 21760 elems per partition
OUT_CSTRIDE = H * W * KPAIRS          # 4096
OUT_NSTRIDE = CPC * OUT_CSTRIDE       # 262144

OP_COEFFS = np.array([
    [0.0, 0.0, 0.0, 0.0], [0.0, 0.0, 0.0, 1.0], [0.0, 1.0, 0.0, -1.0],
    [0.0, 1.0, 0.0, 0.0], [0.0, 0.0, 1.0, -1.0], [0.0, 0.0, 1.0, 0.0],
    [0.0, 1.0, 1.0, -2.0], [0.0, 1.0, 1.0, -1.0], [1.0, -1.0, -1.0, 1.0],
    [1.0, -1.0, -1.0, 2.0], [1.0, 0.0, -1.0, 0.0], [1.0, 0.0, -1.0, 1.0],
    [1.0, -1.0, 0.0, 0.0], [1.0, -1.0, 0.0, 1.0], [1.0, 0.0, 0.0, -1.0],
    [1.0, 0.0, 0.0, 0.0],
], dtype=np.float64)

MULT = mybir.AluOpType.mult
ADD = mybir.AluOpType.add
COPY = mybir.ActivationFunctionType.Copy

# Cost-model ns for load balancing (f32, [128, 256] tiles)
DVE_TT = 327.0   # tensor_tensor, 1x
DVE_TS = 194.0   # tensor_scalar, 2x_2P
ACT_TS = 507.0   # activation, 1x + 352cyc overhead
GPS_TS = 600.0   # gpsimd tensor_scalar (sw impl efficiency ~0.6)
GPS_TT = 600.0   # gpsimd tensor_tensor (sw impl efficiency ~0.42)
PE_U = 852.0     # two f32 matmuls (identity copy + bias row) on TensorE

# Tuning knobs (A/B'd via TimelineSim; best found = ~166us slowest core)
CFG = {
    "use_gps": True,     # offload v/y ops to GPSIMD
    "tp_bufs": 6,
    "yc_bufs": 8,
    "u_act_only": True,  # u always on ScalarE
    "w_dve_only": True,  # w always on VectorE
    "load_chunk_ch": 32,  # 2-chunk load (top-used channels first)
    "load_cascade": None,
    "any_uv": False,
    # TensorE u-offload (identity matmul + bias row, u = B + ka/kab, kab
    # re-applied at the y STT): numerically exact but model-NEGATIVE -- f32
    # matmul runs at 4 cycles/row plus cold p-state, and the PE->PSUM->DVE
    # chain serializes; every tested fraction lost ~7us. Kept for reference.
    "u_pe": False,
    "kab_min": 1e-3,     # |kab| guard for the u_pe refactoring
}

last_results = [None] * N_CORES  # BassKernelResults per core (for profiling)
last_model_ns = [None] * N_CORES  # per-core TimelineSim estimate of the shipped program


def build_core_program(core, ch, ry, rx, coef):
    """One specialized Bass program for `core` (channels core*CPC..+CPC)."""
    nc = bacc.Bacc("TRN2", target_bir_lowering=False)
    xh_d = nc.dram_tensor("xh", [P, XFREE], mybir.dt.float32, kind="ExternalInput")
    kap_d = nc.dram_tensor(
        "kap", [P, CPC * KPAIRS], mybir.dt.float32, kind="ExternalInput"
    )
    out_d = nc.dram_tensor(
        "out", [N, CPC, H, W, KPAIRS], mybir.dt.float32, kind="ExternalOutput"
    )

    use_gps = CFG["use_gps"]
    use_pe = CFG.get("u_pe")
    kab_min = CFG.get("kab_min", 1e-3)
    eng_ns = {"dve": 0.0, "act": 0.0, "gps": 0.0, "pe": 0.0}

    with TileContext(nc) as tc:
        with (
            tc.tile_pool(name="xp", bufs=1) as xpool,
            tc.tile_pool(name="tp", bufs=CFG["tp_bufs"]) as tpool,
            tc.tile_pool(name="yp", bufs=CFG["yc_bufs"]) as ypool,
            tc.tile_pool(name="pp", bufs=4, space="PSUM") as ppool,
        ):
            xh = xpool.tile([P, XFREE], mybir.dt.float32)
            # Jointly order output-channel processing (greedy: next output
            # needing fewest not-yet-loaded inputs) and stream input-channel
            # loads in that discovery order, so compute starts after a couple
            # of small chunks and fully overlaps the rest of the load.
            # (Tile's subtile tracking scopes each pair's waits to the load
            # DMAs it actually reads; Bacc splits any multi-wait syncs.)
            # Two-chunk load: chunk1 = the 32 most-used input channels, then
            # the rest.  A pair only needs its 2 input channels, so ~25-35%
            # of pairs are ready after chunk1 (~16us in); channels are ordered
            # by ready-pair count and ready pairs emitted first, so compute
            # overlaps the chunk2 load.  (Tile's subtile tracking scopes each
            # pair's waits to the load DMAs it reads.)
            chunk_ch = CFG.get("load_chunk_ch", 0)
            pair_chs = {
                (cl, p4): (int(ch[core * CPC + cl, 2 * p4]), int(ch[core * CPC + cl, 2 * p4 + 1]))
                for cl in range(CPC)
                for p4 in range(KPAIRS)
            }
            if chunk_ch <= 0:
                cl_order = list(range(CPC))
                p4_order = {cl: list(range(KPAIRS)) for cl in range(CPC)}
                nc.sync.dma_start(xh[:], xh_d[:])
            else:
                use_cnt = [0] * C_IN
                for a, b in pair_chs.values():
                    use_cnt[a] += 1
                    use_cnt[b] += 1
                by_use = sorted(range(C_IN), key=lambda i: -use_cnt[i])
                sizes = CFG.get("load_cascade") or [chunk_ch, C_IN - chunk_ch]
                tier_of = {}
                pos = 0
                groups = []
                for t, sz in enumerate(sizes):
                    grp = by_use[pos : pos + sz]
                    pos += sz
                    for cch in grp:
                        tier_of[cch] = t
                    if grp:
                        groups.append(sorted(grp))
                ptier = {
                    (cl, p4): max(tier_of[a], tier_of[b])
                    for (cl, p4), (a, b) in pair_chs.items()
                }
                tiers = {cl: sorted(ptier[(cl, p4)] for p4 in range(KPAIRS)) for cl in range(CPC)}
                cl_order = sorted(range(CPC), key=lambda cl: tiers[cl])
                p4_order = {
                    cl: sorted(range(KPAIRS), key=lambda p4: ptier[(cl, p4)])
                    for cl in range(CPC)
                }
                for grp in groups:
                    run = [grp[0]]
                    for cch in grp[1:] + [None]:
                        if cch is not None and cch == run[-1] + 1:
                            run.append(cch)
                            continue
                        lo, hi = run[0] * CHSZ, (run[-1] + 1) * CHSZ
                        nc.sync.dma_start(xh[:, lo:hi], xh_d[:, lo:hi])
                        if cch is not None:
                            run = [cch]
            base = xh[:]
            pitch = base.ap[0][0]
            tens = base.tensor
            base_off = base.offset

            if use_pe:
                ident = xpool.tile([P, P], mybir.dt.float32, tag="ident")
                ones = xpool.tile([1, W * OYS], mybir.dt.float32, tag="ones")
                kap = xpool.tile([P, CPC * KPAIRS], mybir.dt.float32, tag="kap")
                make_identity(nc, ident[:])
                nc.vector.memset(ones[:], 1.0)
                nc.sync.dma_start(kap[:], kap_d[:])

            for cl in cl_order:
                c = core * CPC + cl
                yc = ypool.tile([P, OYS * W * KPAIRS], mybir.dt.float32, tag="yc")
                ybase = yc[:]
                ypitch = ybase.ap[0][0]
                for p4 in p4_order[cl]:
                    ka_, kb_ = 2 * p4, 2 * p4 + 1
                    offA = base_off + int(ch[c, ka_]) * CHSZ + int(ry[c, ka_]) * W34 + int(rx[c, ka_])
                    offB = base_off + int(ch[c, kb_]) * CHSZ + int(ry[c, kb_]) * W34 + int(rx[c, kb_])
                    A_ap = bass.AP(tens, offA, [[pitch, P], [W34, OYS], [1, W]])
                    B_ap = bass.AP(tens, offB, [[pitch, P], [W34, OYS], [1, W]])

                    k0 = float(coef[c, p4, 0])
                    ka = float(coef[c, p4, 1])
                    kb = float(coef[c, p4, 2])
                    kab = float(coef[c, p4, 3])

                    u = tpool.tile([P, OYS * W], mybir.dt.float32, tag="u")
                    v = tpool.tile([P, OYS * W], mybir.dt.float32, tag="v")
                    w = tpool.tile([P, OYS * W], mybir.dt.float32, tag="w")
                    u3 = u[:].rearrange("p (a b) -> p a b", b=W)
                    v3 = v[:].rearrange("p (a b) -> p a b", b=W)
                    w3 = w[:].rearrange("p (a b) -> p a b", b=W)

                    def pick(cands):
                        eng, cost = min(cands, key=lambda c: eng_ns[c[0]] + c[1])
                        eng_ns[eng] += cost
                        return eng

                    # u = kab*B + ka
                    pe_ok = use_pe and abs(kab) >= kab_min
                    ueng = None
                    if CFG.get("any_uv"):
                        nc.any.tensor_scalar(u3, B_ap, kab, ka, MULT, ADD)
                        nc.any.tensor_scalar(v3, B_ap, kb, k0, MULT, ADD)
                        eng_ns["act"] += ACT_TS  # rough accounting
                        eng_ns["dve"] += DVE_TS
                    else:
                        if CFG.get("u_act_only"):
                            ucands = [("act", ACT_TS)]
                        else:
                            ucands = [("act", ACT_TS), ("dve", DVE_TS)]
                            if use_gps:
                                ucands.append(("gps", GPS_TS))
                        if pe_ok:
                            ucands = ucands + [("pe", PE_U)]
                        ueng = pick(ucands)
                        if ueng == "pe":
                            # u = B + ka/kab via identity matmul + bias row;
                            # kab is re-applied at the y step (STT below).
                            upsum = ppool.tile([P, OYS * W], mybir.dt.float32, tag="up")
                            j = cl * KPAIRS + p4
                            nc.tensor.matmul(
                                out=upsum[:], lhsT=ident[:], rhs=B_ap,
                                start=True, stop=False,
                            )
                            nc.tensor.matmul(
                                out=upsum[:],
                                lhsT=kap[0:1, j : j + 1].to_broadcast((1, P)),
                                rhs=ones[:], start=False, stop=True,
                            )
                            u3 = upsum[:].rearrange("p (a b) -> p a b", b=W)
                        elif ueng == "act":
                            nc.scalar.activation(u3, B_ap, COPY, bias=ka, scale=kab)
                        elif ueng == "gps":
                            nc.gpsimd.tensor_scalar(u3, B_ap, kab, ka, MULT, ADD)
                        else:
                            nc.vector.tensor_scalar(u3, B_ap, kab, ka, MULT, ADD)
                        # v = kb*B + k0
                        vcands = [("dve", DVE_TS), ("act", ACT_TS)]
                        if use_gps:
                            vcands.append(("gps", GPS_TS))
                        veng = pick(vcands)
                        if veng == "act":
                            nc.scalar.activation(v3, B_ap, COPY, bias=k0, scale=kb)
                        elif veng == "gps":
                            nc.gpsimd.tensor_scalar(v3, B_ap, kb, k0, MULT, ADD)
                        else:
                            nc.vector.tensor_scalar(v3, B_ap, kb, k0, MULT, ADD)
                    # w = u * A
                    wcands = [("dve", DVE_TT)]
                    if use_gps and not CFG.get("w_dve_only"):
                        wcands.append(("gps", GPS_TT))
                    weng = pick(wcands)
                    if weng == "gps":
                        nc.gpsimd.tensor_tensor(w3, u3, A_ap, MULT)
                    else:
                        nc.vector.tensor_tensor(w3, u3, A_ap, MULT)
                    # y = w + v, written p-interleaved into yc
                    yap = bass.AP(
                        ybase.tensor, ybase.offset + p4,
                        [[ypitch, P], [W * KPAIRS, OYS], [KPAIRS, W]],
                    )
                    ycands = [("dve", DVE_TT)]
                    if use_gps:
                        ycands.append(("gps", GPS_TT))
                    yeng = pick(ycands)
                    if not CFG.get("any_uv") and ueng == "pe":
                        # y = kab*w + v (kab deferred from the PE u form)
                        if yeng == "gps":
                            nc.gpsimd.scalar_tensor_tensor(yap, w3, kab, v3, MULT, ADD)
                        else:
                            nc.vector.scalar_tensor_tensor(yap, w3, kab, v3, MULT, ADD)
                    elif yeng == "gps":
                        nc.gpsimd.tensor_tensor(yap, w3, v3, ADD)
                    else:
                        nc.vector.tensor_tensor(yap, w3, v3, ADD)

                # DMA this channel out: HBM [n, oyblk, (oy',ox,p)=1024]
                oap = bass.AP(
                    out_d, cl * OUT_CSTRIDE,
                    [[OUT_NSTRIDE, N], [OYS * W * KPAIRS, OYB], [1, OYS * W * KPAIRS]],
                )
                nc.sync.dma_start(oap, yc[:])
    nc.finalize()  # Bacc: splits >1-wait syncs into event semaphores
    return nc


def _prep_inputs(x, weights, selection):
    x = np.ascontiguousarray(np.asarray(x, dtype=np.float32))
    weights = np.asarray(weights, dtype=np.float32)
    selection = np.asarray(selection, dtype=np.int32)

    # coefficients: softmax over 16 logic ops folded into {1,a,b,ab} basis
    w64 = weights.astype(np.float64)
    e = np.exp(w64 - w64.max(axis=-1, keepdims=True))
    prob = e / e.sum(axis=-1, keepdims=True)
    coef = (prob @ OP_COEFFS).astype(np.float32)  # [C_OUT, 4, 4]

    ch = ((selection >> 16) & 0xFFFF).astype(np.int64)
    ry = ((selection >> 8) & 0xFF).astype(np.int64)
    rx = (selection & 0xFF).astype(np.int64)

    # halo layout: xh[q=(n,oyblk), ch, r, w] = xpad[n, ch, oyblk*8+r, w]
    xpad = np.zeros((N, C_IN, H + 2, W + 2), dtype=np.float32)
    xpad[:, :, 1 : H + 1, 1 : W + 1] = x
    xh = np.empty((N, OYB, C_IN, HALO, W34), dtype=np.float32)
    for b in range(OYB):
        xh[:, b] = xpad[:, :, b * OYS : b * OYS + HALO, :]
    xh = np.ascontiguousarray(xh.reshape(P, XFREE))
    return xh, ch, ry, rx, coef


def kernel(x, weights, selection):
    assert x.shape == (N, C_IN, H, W), x.shape
    assert weights.shape == (C_OUT, 4, 16), weights.shape
    assert selection.shape == (C_OUT, 8), selection.shape

    xh, ch, ry, rx, coef = _prep_inputs(x, weights, selection)

    # per-core ka/kab bias rows for the TensorE u-path (0 where unused)
    kab_min = CFG.get("kab_min", 1e-3)
    kap_arrs = []
    for k in range(N_CORES):
        kap = np.zeros((P, CPC * KPAIRS), dtype=np.float32)
        for cl in range(CPC):
            c = k * CPC + cl
            for p4 in range(KPAIRS):
                kab_v = float(coef[c, p4, 3])
                if abs(kab_v) >= kab_min:
                    kap[:, cl * KPAIRS + p4] = float(coef[c, p4, 1]) / kab_v
        kap_arrs.append(kap)

    # Per-core auto-tune: each core's selection pattern favors a different
    # chunk1 size for the load/compute overlap — build a few candidates and
    # keep the one the TimelineSim cost model scores fastest.
    try:
        from concourse.timeline_sim import TimelineSim
    except Exception:  # noqa: BLE001
        TimelineSim = None
    cands = CFG.get(
        "tune_candidates",
        (
            (32, 507.0, 600.0), (36, 507.0, 600.0), (40, 507.0, 600.0),
            (44, 507.0, 600.0), (36, 480.0, 600.0), (40, 480.0, 600.0),
            (40, 480.0, 500.0), (36, 480.0, 500.0), (38, 480.0, 500.0),
            (40, 480.0, 550.0),
        ),
    )
    progs = []
    base_chunk = CFG["load_chunk_ch"]
    global ACT_TS, GPS_TS
    base_act, base_gts = ACT_TS, GPS_TS
    for k in range(N_CORES):
        best = None
        for cc, act, gts in cands if TimelineSim is not None else ((base_chunk, base_act, base_gts),):
            CFG["load_chunk_ch"] = cc
            ACT_TS, GPS_TS = act, gts
            nc = build_core_program(k, ch, ry, rx, coef)
            ns = None
            if TimelineSim is not None:
                try:
                    ns = TimelineSim(nc, trace=False).simulate()
                except Exception:  # noqa: BLE001
                    ns = None
            if best is None or (ns is not None and best[0] is not None and ns < best[0]):
                best = (ns, nc)
            if ns is None:
                break
        progs.append(best[1])
        last_model_ns[k] = best[0]
    CFG["load_chunk_ch"] = base_chunk
    ACT_TS, GPS_TS = base_act, base_gts

    import jax

    devices = jax.devices()
    assert len(devices) >= N_CORES, devices

    outs = [None] * N_CORES
    errs = [None] * N_CORES
    # NTFF tracing needs axon hooks that aren't present in this container —
    # make sure run_bass_kernel_spmd never tries (BASS_TRACE in env would).
    os.environ["BASS_NEVER_TRACE"] = "1"

    def run_one(k):
        try:
            with jax.default_device(devices[k]):
                res = bass_utils.run_bass_kernel_spmd(
                    progs[k], [{"xh": xh, "kap": kap_arrs[k]}], core_ids=[k]
                )
            last_results[k] = res
            outs[k] = res.results[0]["out"]
        except Exception as e:  # noqa: BLE001
            errs[k] = e

    threads = [threading.Thread(target=run_one, args=(k,)) for k in range(N_CORES)]
    for t in threads:
        t.start()
    for t in threads:
        t.join()
    for k, e in enumerate(errs):
        if e is not None:
            raise RuntimeError(f"core {k} failed") from e

    y = np.empty((N, C_OUT, H, W, KPAIRS), dtype=np.float32)
    for k in range(N_CORES):
        y[:, k * CPC : (k + 1) * CPC] = outs[k]
    return y
